# revision 16
# baseline (speedup 1.0000x reference)
"""Trainium2 Bass kernel for ConditionalLinearAttention.

Math (per batch element b, shapes hardcoded):
  xf  = x[b].reshape(256, 4096)
  cf  = cond_emb[b].reshape(512, 128)
  kv  = Wcond @ cf                      # (1024, 128)
  k   = softmax(kv[:512], per-row over the 128 cond positions)
  v   = kv[512:]
  ctx[h] = k_h @ v_h.T                  # (64, 64) per head h
  out = Wout @ apply(ctx) @ Wq @ xf + b_out

The whole attention folds into one per-batch matrix
W_comb = Wout @ ctxE @ Wq (256x256); the spatial dimension then sees ONE
(256x256)@(256x4096) GEMM. Sharding: data-parallel over batch, one batch
element per core.

The kernel is HBM-byte-bound and DMA-issue-bound (a dma_start costs
~0.6us on its issuing engine), so all I/O is bf16, DMAs are few and
large, and every DMA row is >=2KB via host-side partition-contiguous
packing:
  pk1a[p, 640j+c]  = [cf | Wcond_k^T] row (j*128+p)       (one DMA)
  pk1b[p, 512j+c]  = Wcond_v^T row (j*128+p)              (one DMA)
  pk2[p, 512j+c]   = [Wq | WoutT] row (j*128+p)           (one DMA)
  x_t[p, 4096ck+n] = xf row (ck*128+p), col n             (8 chunk DMAs)
  out[p, 2048w+1024mo+n'] = OUT row (mo*128+p)            (4 wave DMAs)
Softmax normalization is folded into the Wq rows (A = ctx^T (Wq/Z)), so
exp stays un-normalized, the context block extraction is 2 plain strided
copies, and no on-chip transpose is needed. k and v projections are one
fused N=1024 matmul group into a 2-bank PSUM tile. Phase 2 runs 4 waves
of 1024 spatial columns overlapping the x chunk stream; W_comb is built
in two ck-halves so the first phase-2 matmuls start before the second
half is cast. Phase-1 elementwise work alternates scalar/vector so the
serial chain stays short; trailing junk matmuls keep the PE HAM clock at
8/8 (2.4 GHz) through the framework teardown.
"""

import os

import numpy as np

B = 8
C = 256
N_SPATIAL = 4096  # 64*64
P = 128
N_CORES = 8

WARM_START = int(os.environ.get("KERNEL_WARM_START", "7"))
WARM_MID = int(os.environ.get("KERNEL_WARM_MID", "0"))
WARM_END = int(os.environ.get("KERNEL_WARM_END", "12"))
FUSED_KV = int(os.environ.get("KERNEL_FUSED_KV", "1"))

_CACHE = {}
LAST_RESULTS = None  # BassKernelResults of the most recent run (for test.py)


def _build_nc():
    import concourse.bacc as bacc
    import concourse.mybir as mybir
    import concourse.tile as tile

    fp32 = mybir.dt.float32
    bf16 = mybir.dt.bfloat16
    AF = mybir.ActivationFunctionType

    nc = bacc.Bacc("TRN2", target_bir_lowering=False, debug=False,
                   num_devices=N_CORES)

    x_t = nc.dram_tensor("x", [P, 2 * N_SPATIAL], bf16, kind="ExternalInput").ap()
    pk1a_t = nc.dram_tensor("pk1a", [P, 4 * 640], bf16, kind="ExternalInput").ap()
    pk1b_t = nc.dram_tensor("pk1b", [P, 4 * 512], bf16, kind="ExternalInput").ap()
    pk2_t = nc.dram_tensor("pk2", [P, 4 * 512], bf16, kind="ExternalInput").ap()
    bias_t = nc.dram_tensor("bias", [P, 2], fp32, kind="ExternalInput").ap()
    out_t = nc.dram_tensor("out", [P, 2 * N_SPATIAL], bf16, kind="ExternalOutput").ap()

    NW = 512          # matmul moving width
    WAVE = 1024       # phase-2 columns per wave
    NWV = N_SPATIAL // WAVE   # 4 waves

    with tile.TileContext(nc) as tc:
        with (
            tc.tile_pool(name="main", bufs=1) as mainp,
            tc.tile_pool(name="work", bufs=2) as workp,
            tc.tile_pool(name="outp", bufs=4) as outp,
            tc.tile_pool(name="ps", bufs=2, space="PSUM") as psp,
            tc.tile_pool(name="psO", bufs=6, space="PSUM") as psO,
        ):
            # out DRAM col = 2048*w + 1024*mo + n'
            outr = out_t.rearrange("p (w mo n) -> p w mo n", w=NWV, mo=2)

            # --- input DMAs, critical-path order on the sync HWDGE ring
            pk1a_sb = []
            for h in range(2):
                t = mainp.tile([P, 2 * 640], bf16, tag=f"pk1a{h}")
                nc.sync.dma_start(t, pk1a_t[:, 1280 * h:1280 * (h + 1)])
                pk1a_sb.append(t)
            pk1b_sb = []
            for h in range(2):
                t = mainp.tile([P, 2 * 512], bf16, tag=f"pk1b{h}")
                nc.sync.dma_start(t, pk1b_t[:, 1024 * h:1024 * (h + 1)])
                pk1b_sb.append(t)
            pk2_sb = mainp.tile([P, 4 * 512], bf16)
            nc.sync.dma_start(pk2_sb, pk2_t)
            x_sb = []
            for cc in range(2 * NWV):   # order: g0ck0, g0ck1, g1ck0, ...
                g, ck = divmod(cc, 2)
                t = mainp.tile([P, WAVE], bf16, tag=f"x{cc}")
                nc.sync.dma_start(
                    t, x_t[:, 4096 * ck + WAVE * g: 4096 * ck + WAVE * (g + 1)])
                x_sb.append(t)
            # bias: 128 x 8B descriptors -> keep off the sync ring
            bias_sb = mainp.tile([P, 2], fp32)
            nc.gpsimd.dma_start(bias_sb, bias_t)

            # warmup operand tiles + small constants (emitted first per engine)
            wl = mainp.tile([P, P], bf16)
            nc.gpsimd.memset(wl, 0.0)
            ctx_bd = mainp.tile([P, 4, P], bf16)
            nc.gpsimd.memset(ctx_bd, 0.0)
            wz = mainp.tile([P, NW], bf16)
            nc.vector.memset(wz, 0.0)
            ones_sb = mainp.tile([P, 1], fp32)
            nc.vector.memset(ones_sb, 1.0)

            def keep_warm(n):
                for _ in range(n):
                    pj = psO.tile([P, NW], fp32, tag="O")
                    nc.tensor.matmul(pj, wl, wz, start=True, stop=True)

            keep_warm(WARM_START)

            # --- phase 1: per-batch W_comb (256x256) ---
            # kvT (cond position m on partitions): k half first (its chain is
            # deeper), v half as soon as pk1b lands
            pkv = psp.tile([P, NW], fp32, tag="p1")
            for j in range(4):
                h, jj = divmod(j, 2)
                nc.tensor.matmul(pkv, pk1a_sb[h][:, 640 * jj:640 * jj + 128],
                                 pk1a_sb[h][:, 640 * jj + 128:640 * (jj + 1)],
                                 start=(j == 0), stop=(j == 3))
            pvv = psp.tile([P, NW], fp32, tag="p1")
            for j in range(4):
                h, jj = divmod(j, 2)
                nc.tensor.matmul(pvv, pk1a_sb[h][:, 640 * jj:640 * jj + 128],
                                 pk1b_sb[h][:, 512 * jj:512 * (jj + 1)],
                                 start=(j == 0), stop=(j == 3))
            ex_src, v_src = pkv, pvv
            expkT = mainp.tile([P, NW], fp32)
            nc.scalar.activation(out=expkT, in_=ex_src, func=AF.Exp)
            vT = mainp.tile([P, NW], fp32)
            nc.vector.tensor_copy(out=vT, in_=v_src)

            # softmax denominators: Z[hd] = sum_m expkT[m, hd]; fold 1/Z into
            # the Wq rows (the A-matmul contracts over hd on partitions)
            pz = psp.tile([P, 4], fp32, tag="p1")
            for i in range(4):
                nc.tensor.matmul(pz[:, i:i + 1], expkT[:, 128 * i:128 * (i + 1)],
                                 ones_sb, start=True, stop=True)
            rc = workp.tile([P, 4], fp32)
            nc.vector.reciprocal(rc, pz)
            wqs = mainp.tile([P, 4, 256], bf16)
            for i in range(2):
                nc.vector.tensor_scalar_mul(wqs[:, i, :],
                                            pk2_sb[:, 512 * i:512 * i + 256],
                                            rc[:, i:i + 1])
            for i in range(2, 4):
                nc.scalar.activation(out=wqs[:, i, :],
                                     in_=pk2_sb[:, 512 * i:512 * i + 256],
                                     func=AF.Identity, bias=0.0,
                                     scale=rc[:, i:i + 1])
            keep_warm(WARM_MID)

            # per-head-pair context: diagonal 64x64 blocks extracted with two
            # strided copies into the zeroed block-diagonal layout
            pc = psp.tile([P, 4, 128], fp32, tag="p1")
            for i in range(4):
                nc.tensor.matmul(pc[:, i, :], expkT[:, 128 * i:128 * (i + 1)],
                                 vT[:, 128 * i:128 * (i + 1)], start=True, stop=True)
            keep_warm(1)
            nc.scalar.activation(out=ctx_bd[0:64, :, 0:64],
                                 in_=pc[0:64, :, 0:64], func=AF.Copy)
            nc.vector.tensor_copy(out=ctx_bd[64:128, :, 64:128],
                                  in_=pc[64:128, :, 64:128])

            keep_warm(2)
            # A[he, c] = blockdiag(ctx).T @ (Wq/Z)  (k-tile i = head pair i)
            paA = psp.tile([P, 2, 256], fp32, tag="p1")
            paB = psp.tile([P, 2, 256], fp32, tag="p1")
            for i in range(4):
                pa = paA[:, i, :] if i < 2 else paB[:, i - 2, :]
                nc.tensor.matmul(pa, ctx_bd[:, i, :], wqs[:, i, :],
                                 start=True, stop=True)
            keep_warm(2)
            A_sb = mainp.tile([P, 4, 256], bf16)
            nc.scalar.activation(out=A_sb[:, 0:2, :], in_=paA, func=AF.Copy)
            nc.vector.tensor_copy(out=A_sb[:, 2:4, :], in_=paB)

            keep_warm(2)
            # W_combT[c, o] = sum_he A[he, c] * WoutT[he, o], split by ck-half
            # so phase 2's ck=0 matmuls start before the ck=1 half is cast
            wc = []
            for mc in range(2):
                pw = psp.tile([P, 256], fp32, tag="p1", name=f"pw{mc}")
                for kk in range(4):
                    nc.tensor.matmul(pw, A_sb[:, kk, 128 * mc:128 * (mc + 1)],
                                     pk2_sb[:, 512 * kk + 256:512 * (kk + 1)],
                                     start=(kk == 0), stop=(kk == 3))
                w_t = mainp.tile([P, 256], bf16, tag=f"wc{mc}")
                if mc == 0:
                    nc.vector.tensor_copy(out=w_t, in_=pw)
                else:
                    nc.scalar.activation(out=w_t, in_=pw, func=AF.Copy)
                wc.append(w_t)

            # --- phase 2: OUT = W_comb @ xf + bias, 4 waves of 1024 columns
            # loop order (mo, ck, sub) reuses the PE stationary operand
            for w in range(NWV):
                xcA, xcB = x_sb[2 * w], x_sb[2 * w + 1]
                ot = outp.tile([P, 2, WAVE], bf16, tag="osb")
                ps = [psO.tile([P, NW], fp32, tag="O", name=f"psO_w{w}_{k}")
                      for k in range(4)]
                for mo in range(2):
                    for ck in range(2):
                        xc = xcA if ck == 0 else xcB
                        for sub in range(2):
                            nc.tensor.matmul(
                                ps[2 * mo + sub],
                                wc[ck][:, 128 * mo:128 * (mo + 1)],
                                xc[:, NW * sub:NW * (sub + 1)],
                                start=(ck == 0), stop=(ck == 1))
                for sub in range(2):
                    nc.scalar.activation(out=ot[:, 0, NW * sub:NW * (sub + 1)],
                                         in_=ps[sub], func=AF.Identity,
                                         bias=bias_sb[:, 0:1], scale=1.0)
                nc.sync.dma_start(outr[:, w, 0:1, :], ot[:, 0:1, :])
                for sub in range(2):
                    nc.vector.tensor_scalar_add(out=ot[:, 1, NW * sub:NW * (sub + 1)],
                                                in0=ps[2 + sub],
                                                scalar1=bias_sb[:, 1:2])
                nc.sync.dma_start(outr[:, w, 1:2, :], ot[:, 1:2, :])

            keep_warm(WARM_END)

    nc.compile()
    return nc


def kernel(x, cond_emb, Wq, Wcond, Wout, b_out):
    from concourse.bass_utils import run_bass_kernel_spmd
    import ml_dtypes

    global LAST_RESULTS

    if "nc" not in _CACHE:
        _CACHE["nc"] = _build_nc()
    nc = _CACHE["nc"]

    bf = ml_dtypes.bfloat16
    # x: (8,256,64,64) -> per-batch [128, 2*4096] partition-contiguous
    xf = np.asarray(x, np.float32).reshape(B, 2, P, N_SPATIAL)
    xp = np.ascontiguousarray(xf.transpose(0, 2, 1, 3)).reshape(B, P, 2 * N_SPATIAL)
    xp = xp.astype(bf)
    # pk1a: [cf | Wcond_k^T] (512, 640); pk1b: Wcond_v^T (512, 512)
    cf = np.asarray(cond_emb, np.float32).reshape(B, 512, 128)
    wcondT = np.ascontiguousarray(np.asarray(Wcond, np.float32).T)  # (512, 1024)
    pk1a = np.empty((B, 512, 640), np.float32)
    pk1a[:, :, 0:128] = cf
    pk1a[:, :, 128:640] = wcondT[None, :, 0:512]
    pk1a = np.ascontiguousarray(
        pk1a.reshape(B, 4, P, 640).transpose(0, 2, 1, 3)).reshape(B, P, 4 * 640)
    pk1a = pk1a.astype(bf)
    pk1b = np.ascontiguousarray(
        wcondT[:, 512:1024].reshape(4, P, 512).transpose(1, 0, 2)
    ).reshape(P, 4 * 512).astype(bf)
    # pk2: [Wq | WoutT] (512, 512) -> [128, 4*512] (same for all cores)
    pk2 = np.concatenate([np.asarray(Wq, np.float32),
                          np.ascontiguousarray(np.asarray(Wout, np.float32).T)],
                         axis=1)
    pk2 = np.ascontiguousarray(
        pk2.reshape(4, P, 512).transpose(1, 0, 2)).reshape(P, 4 * 512).astype(bf)
    # bias[p, mo] = b_out[mo*128+p]
    bias = np.ascontiguousarray(np.asarray(b_out, np.float32).reshape(2, P).T)

    in_maps = [
        {
            "x": np.ascontiguousarray(xp[b]),
            "pk1a": np.ascontiguousarray(pk1a[b]),
            "pk1b": pk1b,
            "pk2": pk2,
            "bias": bias,
        }
        for b in range(B)
    ]

    trace = bool(int(os.environ.get("KERNEL_TRACE", "0")))
    res = run_bass_kernel_spmd(nc, in_maps, core_ids=list(range(N_CORES)),
                               trace=trace)
    LAST_RESULTS = res
    # out[p, 2048*w + 1024*mo + n'] = OUT[mo*128+p, 1024*w+n']
    out = np.stack([np.asarray(res.results[b]["out"]) for b in range(B)])
    out = out.reshape(B, P, 4, 2, 1024).transpose(0, 3, 1, 2, 4)
    return np.ascontiguousarray(out).reshape(B, C, 64, 64).astype(np.float32)


if __name__ == "__main__":
    xs = np.random.RandomState(0)
    ins = {
        "x": xs.randn(8, 256, 64, 64).astype(np.float32),
        "cond_emb": xs.randn(8, 512, 1, 128).astype(np.float32),
        "Wq": (xs.randn(512, 256) * 0.05).astype(np.float32),
        "Wcond": (xs.randn(1024, 512) * 0.05).astype(np.float32),
        "Wout": (xs.randn(256, 512) * 0.05).astype(np.float32),
        "b_out": np.zeros(256, np.float32),
    }
    o = kernel(**ins)
    print("ran, shape", o.shape)
